# revision 5
# baseline (speedup 1.0000x reference)
"""Trainium2 Bass kernel for nn_Encoder (6-layer transformer encoder).

Sharding: 8 cores = 4 sequences (batch) x 2 query-halves (sequence split).
Each core computes all 16 heads + full FFN for its 256 "own" query tokens;
K/V projections are computed for the full 512-token sequence on both cores
of a pair (cheap duplication). The only cross-core communication is one
2-rank AllGather of the layer output (fp16, 0.5MB per rank) per layer.

Token order per core is LOCAL: [own 256 | partner 256]. Attention is
permutation-invariant over keys; the host un-permutes the attention-prob
output columns when assembling the full result.

Precision: fp16 operands for Q/K/W1/W2 matmuls, bf16 for V/Wo/exp path
(exp needs bf16 range), fp32 PSUM accumulation, softmax, LayerNorm, and
both outputs. 1/sqrt(dk) is folded into Wq on the host.
"""

import os
import sys

import numpy as np

sys.path.insert(0, "/opt/trn_rl_repo")

# ---- model dims (hardcoded per problem spec) ----
D = 1024
H = 16
DH = 64
DFF = 4096
L = 6
B = 4
S = 512
TQ = 256  # own query tokens per core
KC = 8  # D // 128
LN_EPS = 1e-5
P = 128


def _split_excess_waits(nc, limit=1):
    """Walrus in this container only encodes 1 sync wait per instruction;
    split extra waits onto preceding NoOps on the same engine queue."""
    import bass_rust
    from concourse import mybir

    n_split = 0
    for f in nc.m.functions:
        for bb in f.blocks:
            insns = bb.instructions
            if not any(
                ins.sync_info is not None and len(ins.sync_info.on_wait) > limit
                for ins in insns
            ):
                continue
            new_list = []
            for ins in insns:
                si = ins.sync_info
                if si is not None and len(si.on_wait) > limit:
                    waits = list(si.on_wait)
                    excess, keep = waits[:-limit], waits[-limit:]
                    for i in range(0, len(excess), limit):
                        nop = bass_rust.InstNoOp(
                            name=f"{ins.name}_ws{n_split}_{i}", ins=[], outs=[]
                        )
                        nop.engine = ins.engine
                        nop.sync_info = mybir.SyncInfo(
                            on_wait=excess[i : i + limit], on_update=[]
                        )
                        new_list.append(nop)
                        n_split += 1
                    ins.sync_info = mybir.SyncInfo(
                        on_wait=keep, on_update=list(si.on_update)
                    )
                new_list.append(ins)
            bb.instructions = new_list
    return n_split


def _layernorm(nc, pool, x, eps_t):
    """In-place LN over last dim of x [P, 2, D] fp32."""
    from concourse import mybir

    for mt in range(2):
        stats = pool.tile([P, 2, 6], mybir.dt.float32, name="ln_stats", tag="ln_stats")
        for sg in range(2):
            nc.vector.bn_stats(
                out=stats[:, sg, :], in_=x[:, mt, sg * 512 : (sg + 1) * 512]
            )
        mv = pool.tile([P, 2], mybir.dt.float32, name="ln_mv", tag="ln_mv")
        nc.vector.bn_aggr(out=mv, in_=stats)
        rstd = pool.tile([P, 1], mybir.dt.float32, name="ln_rstd", tag="ln_rstd")
        nc.scalar.activation(
            out=rstd,
            in_=mv[:, 1:2],
            func=mybir.ActivationFunctionType.Sqrt,
            bias=eps_t,
        )
        nc.vector.reciprocal(out=rstd, in_=rstd)
        nc.vector.tensor_scalar(
            out=x[:, mt, :],
            in0=x[:, mt, :],
            scalar1=mv[:, 0:1],
            scalar2=rstd,
            op0=mybir.AluOpType.subtract,
            op1=mybir.AluOpType.mult,
        )


def build_nc(n_layers=L, masked=False):
    import contextlib

    import concourse.bass as bass
    import concourse.tile as tile
    from concourse import mybir
    from concourse.masks import make_identity

    f32 = mybir.dt.float32
    f16 = mybir.dt.float16
    bf16 = mybir.dt.bfloat16

    nc = bass.Bass()

    # ---- DRAM parameters (per-core shards) ----
    xt16_in = nc.declare_dram_parameter("xt16", [D, S], f16, isOutput=False)
    x_own_in = nc.declare_dram_parameter("x_own", [TQ, D], f32, isOutput=False)
    wq_d = nc.declare_dram_parameter("wq", [n_layers, D, D], f16, isOutput=False)
    wk_d = nc.declare_dram_parameter("wk", [n_layers, D, D], f16, isOutput=False)
    wv_d = nc.declare_dram_parameter("wv", [n_layers, D, D], bf16, isOutput=False)
    wo_d = nc.declare_dram_parameter("wo", [n_layers, D, D], bf16, isOutput=False)
    w1_d = nc.declare_dram_parameter("w1", [n_layers, D, DFF], f16, isOutput=False)
    w2_d = nc.declare_dram_parameter("w2", [n_layers, DFF, D], f16, isOutput=False)
    maskcol_d = nc.declare_dram_parameter("maskcol", [4, P, 1], f32, isOutput=False)
    maskrow_d = nc.declare_dram_parameter("maskrow", [P, S], f32, isOutput=False)
    agsel_d = nc.declare_dram_parameter("agsel", [P, 2], f32, isOutput=False)

    xout_d = nc.declare_dram_parameter("xout", [TQ, D], f32, isOutput=True)
    corrs_d = nc.declare_dram_parameter(
        "corrs", [n_layers, H, TQ, S], f32, isOutput=True
    )

    with tile.TileContext(nc) as tc:
        ctx = contextlib.ExitStack()
        with ctx:
            dram = ctx.enter_context(tc.tile_pool(name="dram", bufs=2, space="DRAM"))
            const = ctx.enter_context(tc.tile_pool(name="const", bufs=1))
            actp = ctx.enter_context(tc.tile_pool(name="actp", bufs=1))
            act2 = ctx.enter_context(tc.tile_pool(name="act2", bufs=2))
            wpool = ctx.enter_context(tc.tile_pool(name="wpool", bufs=1))
            wpool2 = ctx.enter_context(tc.tile_pool(name="wpool2", bufs=2))
            smallp = ctx.enter_context(tc.tile_pool(name="smallp", bufs=4))
            psum = ctx.enter_context(tc.tile_pool(name="psum", bufs=4, space="PSUM"))
            psum2 = ctx.enter_context(tc.tile_pool(name="psum2", bufs=2, space="PSUM"))

            # ---- constants ----
            id_f32 = const.tile([P, P], f32, name="id_f32", tag="id_f32")
            make_identity(nc, id_f32)
            id_bf16 = const.tile([P, P], bf16, name="id_bf16", tag="id_bf16")
            make_identity(nc, id_bf16)
            eps_t = const.tile([P, 1], f32, name="eps_t", tag="eps_t")
            nc.vector.memset(eps_t, LN_EPS)
            maskcol = const.tile([P, 4], f32, name="maskcol", tag="maskcol")
            nc.sync.dma_start(
                out=maskcol, in_=maskcol_d.rearrange("c p one -> p (c one)")
            )
            agsel = const.tile([P, 2], f32, name="agsel", tag="agsel")
            nc.sync.dma_start(out=agsel, in_=agsel_d[:])
            maskrow = None
            if masked:
                maskrow = const.tile([P, S], f32, name="maskrow", tag="maskrow")
                nc.sync.dma_start(out=maskrow, in_=maskrow_d[:])

            # ---- initial activations ----
            xt16 = act2.tile([P, KC, S], f16, name="xt16", tag="xt16", bufs=1)
            nc.sync.dma_start(out=xt16, in_=xt16_in.rearrange("(c p) t -> p c t", p=P))
            xtbf = act2.tile([P, KC, S], bf16, name="xtbf", tag="xtbf", bufs=1)
            nc.vector.tensor_copy(out=xtbf, in_=xt16)
            x_own = act2.tile([P, 2, D], f32, name="x_own", tag="xresid")
            nc.sync.dma_start(out=x_own, in_=x_own_in.rearrange("(m p) d -> p m d", p=P))

            groups = [[0, 1], [2, 3], [4, 5], [6, 7]]

            for l in range(n_layers):
                # ============ Phase A: load weights, Q/K/V ============
                wq = wpool.tile([P, KC, D], f16, name="wq_sb", tag="wq_sb")
                nc.sync.dma_start(out=wq, in_=wq_d[l].rearrange("(c p) d -> p c d", p=P))
                wk = wpool.tile([P, KC, D], f16, name="wk_sb", tag="wk_sb")
                nc.sync.dma_start(out=wk, in_=wk_d[l].rearrange("(c p) d -> p c d", p=P))
                wv = wpool.tile([P, KC, D], bf16, name="wv_sb", tag="wv_sb")
                nc.sync.dma_start(out=wv, in_=wv_d[l].rearrange("(c p) d -> p c d", p=P))
                wo = wpool.tile([P, KC, D], bf16, name="wo_sb", tag="wo_sb")
                nc.sync.dma_start(out=wo, in_=wo_d[l].rearrange("(c p) d -> p c d", p=P))

                qt = actp.tile([P, KC, TQ], f16, name="qt", tag="qt")
                kt = actp.tile([P, KC, S], f16, name="kt", tag="kt")
                vt = actp.tile([P, 4, D], bf16, name="vt", tag="vt")

                # Q: qT [hdh(8 tiles), 256]
                for mt in range(KC):
                    ps = psum.tile([P, TQ], f32, tag="ps512", name="ps_q")
                    for kc in range(KC):
                        nc.tensor.matmul(
                            ps,
                            wq[:, kc, mt * P : (mt + 1) * P],
                            xt16[:, kc, 0:TQ],
                            start=(kc == 0),
                            stop=(kc == KC - 1),
                        )
                    nc.scalar.copy(out=qt[:, mt, :], in_=ps)
                # K: kT [hdh, 512]
                for mt in range(KC):
                    ps = psum.tile([P, S], f32, tag="ps512", name="ps_k")
                    for kc in range(KC):
                        nc.tensor.matmul(
                            ps,
                            wk[:, kc, mt * P : (mt + 1) * P],
                            xt16[:, kc, :],
                            start=(kc == 0),
                            stop=(kc == KC - 1),
                        )
                    nc.scalar.copy(out=kt[:, mt, :], in_=ps)
                # V token-major: [tok(4 tiles), hdh 1024]
                for t in range(4):
                    for nb in range(2):
                        ps = psum.tile([P, 512], f32, tag="ps512", name="ps_v")
                        for kc in range(KC):
                            nc.tensor.matmul(
                                ps,
                                xtbf[:, kc, t * P : (t + 1) * P],
                                wv[:, kc, nb * 512 : (nb + 1) * 512],
                                start=(kc == 0),
                                stop=(kc == KC - 1),
                            )
                        nc.vector.tensor_copy(
                            out=vt[:, t, nb * 512 : (nb + 1) * 512], in_=ps
                        )

                # ============ Phase B: attention per head ============
                avn = actp.tile([P, KC, TQ], bf16, name="avn", tag="avn")
                recips = smallp.tile([P, 2, H], f32, name="recips", tag="recips", bufs=2)
                for h in range(H):
                    hc, base = h // 2, (h % 2) * DH
                    # scoresT [Tk(4 tiles), TQ] ; exp -> bf16
                    expT = actp.tile([P, 4, TQ], bf16, name="expT", tag="expT", bufs=2)
                    for c in range(4):
                        ps = psum.tile([P, TQ], f32, tag="ps512", name="ps_sT")
                        nc.tensor.matmul(
                            ps,
                            kt[base : base + DH, hc, c * P : (c + 1) * P],
                            qt[base : base + DH, hc, :],
                            start=True,
                            stop=True,
                        )
                        nc.scalar.activation(
                            out=expT[:, c, :],
                            in_=ps,
                            func=mybir.ActivationFunctionType.Exp,
                            bias=maskcol[:, c : c + 1],
                        )
                    # scores1 [TQ(2 tiles), S]; exp + rowsum; normalize; corrs out
                    for mt in range(2):
                        ps = psum.tile([P, S], f32, tag="ps512", name="ps_s1")
                        nc.tensor.matmul(
                            ps,
                            qt[base : base + DH, hc, mt * P : (mt + 1) * P],
                            kt[base : base + DH, hc, :],
                            start=True,
                            stop=True,
                        )
                        if masked:
                            nc.vector.tensor_add(out=ps, in0=ps, in1=maskrow)
                        exp1 = actp.tile([P, S], f32, name="exp1", tag="exp1", bufs=3)
                        rs = smallp.tile([P, 1], f32, name="rs", tag="rs")
                        nc.scalar.activation(
                            out=exp1,
                            in_=ps,
                            func=mybir.ActivationFunctionType.Exp,
                            accum_out=rs,
                        )
                        rc = recips[:, mt, h : h + 1]
                        nc.vector.reciprocal(out=rc, in_=rs)
                        nc.vector.tensor_scalar_mul(out=exp1, in0=exp1, scalar1=rc)
                        nc.sync.dma_start(
                            out=corrs_d[l, h, mt * P : (mt + 1) * P, :], in_=exp1
                        )
                    # AV: av token-major [TQ(2 tiles), 64], normalize, transpose
                    for t in range(2):
                        ps = psum2.tile([P, DH], f32, tag="ps128", name="ps_av")
                        for c in range(4):
                            nc.tensor.matmul(
                                ps,
                                expT[:, c, t * P : (t + 1) * P],
                                vt[:, c, h * DH : (h + 1) * DH],
                                start=(c == 0),
                                stop=(c == 3),
                            )
                        av_sb = smallp.tile([P, DH], bf16, name="av_sb", tag="av_sb")
                        nc.vector.tensor_scalar_mul(
                            out=av_sb, in0=ps, scalar1=recips[:, t, h : h + 1]
                        )
                        ps_tr = psum2.tile([DH, P], bf16, tag="ps128", name="ps_avT")
                        nc.tensor.transpose(ps_tr, av_sb, id_bf16)
                        nc.scalar.copy(
                            out=avn[base : base + DH, hc, t * P : (t + 1) * P],
                            in_=ps_tr,
                        )

                # ============ Phase C: Wo + residual + LN1 ============
                x1 = act2.tile([P, 2, D], f32, name="x1", tag="xresid")
                for mt in range(2):
                    for nb in range(2):
                        ps = psum.tile([P, 512], f32, tag="ps512", name="ps_o")
                        for hc in range(KC):
                            nc.tensor.matmul(
                                ps,
                                avn[:, hc, mt * P : (mt + 1) * P],
                                wo[:, hc, nb * 512 : (nb + 1) * 512],
                                start=(hc == 0),
                                stop=(hc == KC - 1),
                            )
                        nc.vector.tensor_add(
                            out=x1[:, mt, nb * 512 : (nb + 1) * 512],
                            in0=ps,
                            in1=x_own[:, mt, nb * 512 : (nb + 1) * 512],
                        )
                _layernorm(nc, smallp, x1, eps_t)
                # x1T fp16 [D(8 tiles), TQ]
                x1t = actp.tile([P, KC, TQ], f16, name="x1t", tag="x1t")
                for mt in range(2):
                    for dc in range(KC):
                        ps_tr = psum2.tile([P, P], f32, tag="ps128", name="ps_x1t")
                        nc.tensor.transpose(
                            ps_tr, x1[:, mt, dc * P : (dc + 1) * P], id_f32
                        )
                        nc.scalar.copy(out=x1t[:, dc, mt * P : (mt + 1) * P], in_=ps_tr)

                # ============ Phase D: FFN1 (y1T = relu(x1 @ W1).T) ============
                y1t = actp.tile([P, 32, TQ], f16, name="y1t", tag="y1t")
                w1v = w1_d[l].rearrange("(c p) f -> p c f", p=P)
                for e in range(8):
                    w1e = wpool2.tile([P, KC, 512], f16, name="w1e", tag="w1e")
                    nc.sync.dma_start(
                        out=w1e, in_=w1v[:, :, e * 512 : (e + 1) * 512]
                    )
                    for fm in range(4):
                        ps = psum.tile([P, TQ], f32, tag="ps512", name="ps_f1")
                        for kc in range(KC):
                            nc.tensor.matmul(
                                ps,
                                w1e[:, kc, fm * P : (fm + 1) * P],
                                x1t[:, kc, :],
                                start=(kc == 0),
                                stop=(kc == KC - 1),
                            )
                        nc.scalar.activation(
                            out=y1t[:, e * 4 + fm, :],
                            in_=ps,
                            func=mybir.ActivationFunctionType.Relu,
                        )

                # ============ Phase E: FFN2 + residual + LN2 ============
                ps_y2 = [
                    [
                        psum.tile([P, 512], f32, tag="ps512", name=f"ps_y2_{mt}_{nb}")
                        for nb in range(2)
                    ]
                    for mt in range(2)
                ]
                for kc in range(32):
                    w2c = wpool2.tile([P, D], f16, name="w2c", tag="w2c", bufs=6)
                    nc.sync.dma_start(out=w2c, in_=w2_d[l, kc * P : (kc + 1) * P, :])
                    for mt in range(2):
                        for nb in range(2):
                            nc.tensor.matmul(
                                ps_y2[mt][nb],
                                y1t[:, kc, mt * P : (mt + 1) * P],
                                w2c[:, nb * 512 : (nb + 1) * 512],
                                start=(kc == 0),
                                stop=(kc == 31),
                            )
                x2 = act2.tile([P, 2, D], f32, name="x2", tag="xresid")
                for mt in range(2):
                    for nb in range(2):
                        nc.vector.tensor_add(
                            out=x2[:, mt, nb * 512 : (nb + 1) * 512],
                            in0=ps_y2[mt][nb],
                            in1=x1[:, mt, nb * 512 : (nb + 1) * 512],
                        )
                _layernorm(nc, smallp, x2, eps_t)

                if l == n_layers - 1:
                    nc.sync.dma_start(
                        out=xout_d.rearrange("(m p) d -> p m d", p=P), in_=x2
                    )
                    x_own = x2
                    continue

                # ============ Phase F: transpose + AllGather ============
                xt16_n = act2.tile([P, KC, S], f16, name="xt16", tag="xt16", bufs=1)
                xtbf_n = act2.tile([P, KC, S], bf16, name="xtbf", tag="xtbf", bufs=1)
                ag_in = dram.tile([D, TQ], f16, name="ag_in", tag="ag_in")
                ag_out = dram.tile([2 * D, TQ], f16, name="ag_out", tag="ag_out")
                for mt in range(2):
                    for dc in range(KC):
                        ps_tr = psum2.tile([P, P], f32, tag="ps128", name="ps_x2t")
                        nc.tensor.transpose(
                            ps_tr, x2[:, mt, dc * P : (dc + 1) * P], id_f32
                        )
                        nc.scalar.copy(
                            out=xt16_n[:, dc, mt * P : (mt + 1) * P], in_=ps_tr
                        )
                        nc.vector.tensor_copy(
                            out=xtbf_n[:, dc, mt * P : (mt + 1) * P], in_=ps_tr
                        )
                for dc in range(KC):
                    nc.sync.dma_start(
                        out=ag_in[dc * P : (dc + 1) * P, :], in_=xt16_n[:, dc, 0:TQ]
                    )
                nc.gpsimd.collective_compute(
                    "AllGather",
                    mybir.AluOpType.bypass,
                    ins=[ag_in.opt()],
                    outs=[ag_out.opt()],
                    replica_groups=groups,
                )
                # partner-half select: DMA both AG halves, blend with host 0/1
                agh = actp.tile([P, 2, KC, TQ], f16, name="agh", tag="agh")
                for g in range(2):
                    for dc in range(KC):
                        nc.sync.dma_start(
                            out=agh[:, g, dc, :],
                            in_=ag_out[g * D + dc * P : g * D + (dc + 1) * P, :],
                        )
                for dc in range(KC):
                    tmp = smallp.tile([P, TQ], f16, name="agtmp", tag="agtmp")
                    nc.vector.tensor_scalar_mul(
                        out=tmp, in0=agh[:, 1, dc, :], scalar1=agsel[:, 1:2]
                    )
                    nc.vector.scalar_tensor_tensor(
                        out=xt16_n[:, dc, TQ:S],
                        in0=agh[:, 0, dc, :],
                        scalar=agsel[:, 0:1],
                        in1=tmp,
                        op0=mybir.AluOpType.mult,
                        op1=mybir.AluOpType.add,
                    )
                    nc.vector.tensor_copy(
                        out=xtbf_n[:, dc, TQ:S], in_=xt16_n[:, dc, TQ:S]
                    )
                xt16, xtbf, x_own = xt16_n, xtbf_n, x2

    _split_excess_waits(nc)
    return nc


# ---------------- host side ----------------

_NC_CACHE = {}


def _get_nc(n_layers, masked):
    key = (n_layers, masked)
    if key not in _NC_CACHE:
        _NC_CACHE[key] = build_nc(n_layers, masked)
    return _NC_CACHE[key]


def prepare_inputs(enc_inputs, word_emb, pos_table, Wq, Wk, Wv, Wo, W1, W2, n_layers=L):
    """Build the 8 per-core input maps. Returns (in_maps, masked)."""
    import ml_dtypes

    bf = ml_dtypes.bfloat16
    enc = np.asarray(enc_inputs)
    x0 = np.asarray(word_emb)[enc] + np.asarray(pos_table)[enc]  # [B,S,D] f32
    x0 = x0.astype(np.float32)

    Wq = np.asarray(Wq)[:n_layers] * np.float32(1.0 / np.sqrt(DH))
    Wk = np.asarray(Wk)[:n_layers]
    Wv = np.asarray(Wv)[:n_layers]
    Wo = np.asarray(Wo)[:n_layers]
    W1 = np.asarray(W1)[:n_layers]
    W2 = np.asarray(W2)[:n_layers]

    wq16 = np.ascontiguousarray(Wq).astype(np.float16)
    wk16 = np.ascontiguousarray(Wk).astype(np.float16)
    wvbf = np.ascontiguousarray(Wv).astype(bf)
    wobf = np.ascontiguousarray(Wo).astype(bf)
    w116 = np.ascontiguousarray(W1).astype(np.float16)
    w216 = np.ascontiguousarray(W2).astype(np.float16)

    masked = bool((enc == 0).any())
    in_maps = []
    for c in range(8):
        b, qh = c // 2, c % 2
        # local token order: [own | partner]
        if qh:
            perm = np.concatenate([np.arange(TQ, S), np.arange(0, TQ)])
        else:
            perm = np.arange(S)
        xl = x0[b][perm]  # [S, D] local order
        mask_bias = np.where(enc[b][perm] == 0, -1e30, 0.0).astype(np.float32)  # [S]
        in_maps.append(
            {
                "xt16": np.ascontiguousarray(xl.T).astype(np.float16),
                "x_own": np.ascontiguousarray(xl[0:TQ]).astype(np.float32),
                "wq": wq16,
                "wk": wk16,
                "wv": wvbf,
                "wo": wobf,
                "w1": w116,
                "w2": w216,
                "maskcol": np.ascontiguousarray(mask_bias.reshape(4, P, 1)),
                "maskrow": np.ascontiguousarray(
                    np.broadcast_to(mask_bias, (P, S))
                ).astype(np.float32),
                "agsel": np.ascontiguousarray(
                    np.broadcast_to(
                        np.eye(2, dtype=np.float32)[1 - qh], (P, 2)
                    )
                ),
            }
        )
    return in_maps, masked


def assemble_outputs(results, n_layers=L):
    """results: list of 8 per-core dicts -> (out [B,S,D], corrs [L,B,H,S,S])."""
    out = np.zeros((B, S, D), np.float32)
    corrs = np.zeros((n_layers, B, H, S, S), np.float32)
    for c in range(8):
        b, qh = c // 2, c % 2
        out[b, qh * TQ : (qh + 1) * TQ] = results[c]["xout"]
        cc = results[c]["corrs"]  # [L, H, TQ, S] local k-order
        corrs[:, b, :, qh * TQ : (qh + 1) * TQ, qh * TQ : (qh + 1) * TQ] = cc[
            :, :, :, 0:TQ
        ]
        corrs[:, b, :, qh * TQ : (qh + 1) * TQ, (1 - qh) * TQ : (2 - qh) * TQ] = cc[
            :, :, :, TQ:S
        ]
    return out, corrs


def kernel(
    enc_inputs,
    word_emb,
    pos_table,
    Wq,
    bq,
    Wk,
    bk,
    Wv,
    bv,
    Wo,
    bo,
    W1,
    b1,
    W2,
    b2,
    d_model=D,
    d_q=DH,
    d_k=DH,
    d_v=DH,
):
    from concourse.bass_utils import run_bass_kernel_spmd

    for bias in (bq, bk, bv, bo, b1, b2):
        assert not np.asarray(bias).any(), "nonzero biases not supported"

    in_maps, masked = prepare_inputs(
        enc_inputs, word_emb, pos_table, Wq, Wk, Wv, Wo, W1, W2
    )
    nc = _get_nc(L, masked)
    res = run_bass_kernel_spmd(nc, in_maps, list(range(8)))
    out, corrs = assemble_outputs(res.results)
    return out, corrs
